# revision 22
# baseline (speedup 1.0000x reference)
"""Trainium2 Bass kernel for nn_Attention_51823075393746.

Self-attention block (SAGAN-style) over x:[16,128,64,64]:
  theta = w_theta @ x            [B, 16, 4096]
  phi   = pool2x2(w_phi @ x)     [B, 16, 1024]
  g     = pool2x2(w_g @ x)       [B, 64, 1024]
  beta  = softmax(theta^T phi)   [B, 4096, 1024]
  out   = gamma * (w_o @ (g @ beta^T)) + x

Sharding: data-parallel over batch, 2 samples per core on 8 cores.

Key structure (v2):
  - The K=16 score matmuls are packed 4-wide into the PE's 32-row groups
    via tile_position=(32i, 0): theta is materialized 4x (replicated
    through a padded projection weight, free), phi is pooled once and
    partition-replicated by 3 SBUF->SBUF DMAs.  The 4 packed matmuls run
    concurrently in the array (~3x measured on HW for K=32 row tiling).
  - exp() is the hard floor: 4.2M elements/sample through the one ACT
    engine at 1 elem/cycle/lane.  Each ACTIVATE spans a full 4-bank
    score pack ([128, 2048]) to amortize the (N+352)-cycle overhead, and
    nothing else is scheduled on ACT.
  - softmax uses a compile-time constant shift (exp(score - K)) and the
    denominator comes from a ones-column inside the accumulating
    g^T @ E matmul; normalization (1/s) is applied to the [64, NC]
    pooled-value tensor o before w_o (it commutes), via a partition-
    broadcast DMA of the reciprocal row.
  - residual is added on-chip from the fp16 copy of x (DVE), so x is
    loaded once (fp16) and out written once: ~6 MB DMA/core vs 18.
  - maxpools split across DVE (phi) and GpSimd (g); PSUM->SBUF copies on
    DVE; w_o matmuls deferred 2 chunks so the PE never waits on the
    reciprocal-broadcast chain.
"""

import sys

for _p in ("/opt/trn_rl_repo",):
    if _p not in sys.path:
        sys.path.insert(0, _p)

import numpy as np

import concourse.bass as bass
import concourse.bacc as bacc
import concourse.mybir as mybir
import concourse.tile as tile

F32 = mybir.dt.float32
F16 = mybir.dt.float16
AF = mybir.ActivationFunctionType
ALU = mybir.AluOpType

B, C, H, W = 16, 128, 64, 64
N = H * W          # 4096 spatial positions
M = N // 4         # 1024 pooled positions
CT = 16            # theta/phi channels (C//8)
CG = 64            # g channels (C//2)
NCORES = 8
NS = B // NCORES   # samples per core
NC = 512           # spatial chunk (free dim of matmuls)
NJ = N // NC       # 8 chunks
KM = M // 128      # 8 m-tiles of pooled positions
PR = NC // 4       # pooled positions produced per chunk (128)
K_SHIFT = 5.0      # constant softmax shift: exp(score - K) keeps fp16 happy
PACKED = True      # 4-way row-group packing of the score matmuls


def build_nc(ns: int = NS) -> bass.Bass:
    nc = bacc.Bacc()
    x16d = nc.dram_tensor("x16", [ns, C, N], F16, kind="ExternalInput")
    wt = nc.dram_tensor("wt16", [C, C], F16, kind="ExternalInput")
    wg = nc.dram_tensor("wg16", [C, C], F16, kind="ExternalInput")
    wo = nc.dram_tensor("wo16", [C, C], F16, kind="ExternalInput")
    onec = nc.dram_tensor("onec", [C, KM, CG], F16, kind="ExternalInput")
    identd = nc.dram_tensor("ident", [CG, CG], F16, kind="ExternalInput")
    out = nc.dram_tensor("out", [ns, C, N], F32, kind="ExternalOutput")

    with tile.TileContext(nc) as tc:
        with (
            tc.tile_pool(name="const", bufs=1) as const,
            tc.tile_pool(name="xp", bufs=2) as xp,
            tc.tile_pool(name="tpg", bufs=2) as tpg,
            tc.tile_pool(name="ep", bufs=3) as ep,
            tc.tile_pool(name="osb", bufs=6) as osb,
            tc.tile_pool(name="rbp", bufs=4) as rbp,
            tc.tile_pool(name="outp", bufs=4) as outp,
            tc.tile_pool(name="ps_att", bufs=1, space="PSUM") as ps_att,
            tc.tile_pool(name="ps_o", bufs=1, space="PSUM") as ps_o,
            tc.tile_pool(name="ps_x", bufs=2, space="PSUM") as ps_x,
        ):
            wg_sb = const.tile([C, C], F16)
            nc.sync.dma_start(wg_sb[:], wg[:])
            wt_sb = const.tile([C, C], F16)
            nc.sync.dma_start(wt_sb[:], wt[:])
            wo_sb = const.tile([C, C], F16)
            nc.sync.dma_start(wo_sb[:], wo[:])
            ident = const.tile([CG, CG], F16)
            nc.sync.dma_start(ident[:], identd[:])
            kbias = const.tile([C, 1], F32)
            nc.vector.memset(kbias[:], -K_SHIFT)

            # per-sample state, built by stage A/B, consumed by stage C
            state = {}
            # 5-bank score ring, rotated manually (slice-level deps): each
            # 4-bank pack leaves 1-2 banks free early so the next pack's
            # matmuls overlap the previous exp instead of serializing on it
            ps5 = ps_att.tile([128, 5, NC], F32, tag="s5", name="ps5")
            pack_ctr = [0]

            def load_x16(b):
                x16 = xp.tile([C, N], F16, tag="x16", name="x16sb")
                for j in range(NJ):
                    nc.gpsimd.dma_start(
                        x16[:, j * NC:(j + 1) * NC],
                        x16d[b][:, j * NC:(j + 1) * NC],
                    )
                return x16

            def proj_g_chunk(b, j):
                """g+phi projection, merged maxpool, and the g^T transpose
                for m-tile j (g cols j*128 are final once chunk j pools)."""
                st = state[b]
                x16 = st["x16"]
                gphi = st["gphi"]
                sl = slice(j * NC, (j + 1) * NC)

                # g (rows 0:64) + phi preimage (rows 64:80; PSUM partition
                # slices must start 32-aligned, so phi rides here, not in
                # the theta matmul).  Rows 80:96 are zero weight filler so
                # the single merged pool reads a 96-partition slice.
                pp2 = ps_x.tile([C, NC], F32, tag="pp", name="pp2")
                nc.tensor.matmul(
                    pp2[:], lhsT=wg_sb[:], rhs=x16[:, sl], start=True, stop=True
                )
                # 2x2 maxpool: chunk j covers h rows 8j..8j+7; pooled cols
                # [PR*j, PR*(j+1)) laid out (r w): r = pooled h row (4),
                # w = pooled w col (32).
                v = pp2[0:96].rearrange(
                    "p (r a w b) -> p r w a b", r=4, a=2, b=2
                )
                po = gphi[0:96, j * PR:(j + 1) * PR].rearrange(
                    "p (r w) -> p r w", r=4
                )
                nc.vector.tensor_reduce(po, v, axis=mybir.AxisListType.XY, op=ALU.max)

            def transpose_chunk(b, k):
                """g^T for m-tile k -> ga[:, k, 0:CG]."""
                st = state[b]
                pt = ps_x.tile([C, NC], F16, tag="pp", name="pt")
                nc.tensor.transpose(
                    pt[:, 0:CG], st["gphi"][0:CG, k * 128:(k + 1) * 128],
                    ident[:],
                )
                nc.vector.tensor_copy(st["ga"][:, k, 0:CG], pt[:, 0:CG])

            def proj_t_chunk(b, j):
                """theta x4 projection (rows 32i:32i+16; other rows 0)."""
                st = state[b]
                sl = slice(j * NC, (j + 1) * NC)
                pp = ps_x.tile([C, NC], F32, tag="pp", name="pp")
                nc.tensor.matmul(
                    pp[:], lhsT=wt_sb[:], rhs=st["x16"][:, sl],
                    start=True, stop=True,
                )
                nc.vector.tensor_copy(st["thph"][:, sl], pp[:])

            def start_sample(b):
                st = state[b] = {}
                st["x16"] = load_x16(b)
                st["thph"] = tpg.tile([C, N], F16, tag="thph", name="thph")
                # rows 0:64 = g, rows 64:80 = pooled phi (row group 2 reads
                # phi from here directly), rows 80:96 zero filler
                st["gphi"] = tpg.tile([96, M], F16, tag="gphi", name="gphi")
                # phi replicas for row groups 0, 1, 3
                st["phi_rep"] = tpg.tile([128, M], F16, tag="phi_rep",
                                         name="phi_rep")
                ga = st["ga"] = tpg.tile([128, KM, 2 * CG], F16, tag="ga", name="ga")
                # cols CG:2CG: ones at col CG (softmax denominator), 0 after
                nc.gpsimd.dma_start(ga[:, :, CG:], onec[:])

            def finish_sample_ab(b):
                """phi partition-replication (after all chunks pooled)."""
                st = state[b]
                for r in (0, 1, 3):
                    nc.gpsimd.dma_start(
                        st["phi_rep"][32 * r:32 * r + CT, :],
                        st["gphi"][CG:CG + CT, :],
                    )

            def scores_pack(b, j, t):
                """One 4-way row-group-packed score pack + its exp."""
                st = state[b]
                thph, gphi, phi_rep = st["thph"], st["gphi"], st["phi_rep"]
                sl = slice(j * NC, (j + 1) * NC)
                e_sb = st["e"][j]
                s0 = (pack_ctr[0] * 4) % 5
                pack_ctr[0] += 1
                slots = [(s0 + i) % 5 for i in range(4)]
                for i in range(4):
                    k = 4 * t + i
                    r = 32 * i if PACKED else 0
                    if PACKED and i == 2:
                        lhsT = gphi[CG:CG + CT, k * 128:(k + 1) * 128]
                    else:
                        lhsT = phi_rep[r:r + CT, k * 128:(k + 1) * 128]
                    nc.tensor.matmul(
                        ps5[:, slots[i], :],
                        lhsT=lhsT,
                        rhs=thph[r:r + CT, sl],
                        start=True,
                        stop=True,
                        tile_position=(r, 0),
                    )
                # exp per contiguous slot run (the ring wrap splits some packs)
                i = 0
                while i < 4:
                    ln = 1
                    while i + ln < 4 and slots[i + ln] == slots[i] + ln:
                        ln += 1
                    nc.scalar.activation(
                        e_sb[:, 4 * t + i:4 * t + i + ln, :],
                        ps5[:, slots[i]:slots[i] + ln, :],
                        AF.Exp, bias=kbias[:],
                    )
                    i += ln

            def consume_o_half(b, j, half):
                """Accumulated g^T @ E matmul for chunk j (split in two
                emission halves so score pack t1 sits between them in the
                PE queue); normalization rides with the second half."""
                st = state[b]
                ga = st["ga"]
                if half == 0:
                    st["po"] = ps_o.tile([C, NC], F32, tag="po", name="po")
                po = st["po"]
                e_sb = st["e"][j]
                ks = range(KM // 2) if half == 0 else range(KM // 2, KM)
                for k in ks:
                    nc.tensor.matmul(
                        po[:],
                        lhsT=ga[:, k, :],
                        rhs=e_sb[:, k, :],
                        start=(k == 0),
                        stop=(k == KM - 1),
                    )
                if half == 0:
                    return
                st["e"].pop(j)
                # rows 0:CG = g @ E, row CG = sum_m E = softmax denominator.
                # The bitwise-NOT seed of the fast reciprocal misreads PSUM
                # on HW -- stage the row through SBUF first.
                srow = rbp.tile([1, NC], F32, tag="srow", name="srow")
                nc.vector.tensor_copy(srow[:], po[CG:CG + 1, :])
                rrow = rbp.tile([1, NC], F32, tag="rrow", name="rrow")
                nc.vector.reciprocal_approx_fast(rrow[:], srow[:])
                rb = rbp.tile([CG, NC], F32, tag="rb", name="rb")
                nc.gpsimd.partition_broadcast(rb[:], rrow[0:1, :])
                # even chunks land at partitions 0:64, odd at 64:128 so a
                # pair of w_o matmuls can run packed in row groups {0,1} and
                # {2,3} concurrently
                r0 = (j % 2) * CG
                o_sb = osb.tile([128, NC], F16, tag="o_sb", name="o_sb")
                nc.vector.tensor_tensor(
                    o_sb[r0:r0 + CG, :], po[0:CG, :], rb[:], ALU.mult
                )
                st.setdefault("o", {})[j] = o_sb

            def emit_wo_pair(b, je):
                """w_o for chunks (je, je+1), packed into PE row groups
                {0,1} / {2,3}; then the residual add and the output DMA."""
                st = state[b]
                pfs = []
                for j in (je, je + 1):
                    r0 = (j % 2) * CG
                    pf = ps_x.tile([C, NC], F32, tag="pp", name="pf")
                    nc.tensor.matmul(
                        pf[:],
                        lhsT=wo_sb[r0:r0 + CG, :],
                        rhs=st["o"].pop(j)[r0:r0 + CG, :],
                        start=True, stop=True,
                        tile_position=(r0, 0),
                    )
                    pfs.append(pf)
                for j, pf in zip((je, je + 1), pfs):
                    sl = slice(j * NC, (j + 1) * NC)
                    o3 = outp.tile([C, NC], F32, tag="o3", name="o3")
                    nc.vector.tensor_tensor(
                        o3[:], pf[:], st["x16"][:, sl], ALU.add
                    )
                    nc.gpsimd.dma_start(out[b][:, sl], o3[:])

            # ---- schedule ----
            # Prepay the exp ACT table load (~2.7us) during startup.
            warm = rbp.tile([C, 1], F32, tag="warm", name="warm")
            nc.scalar.activation(warm[:], kbias[:], AF.Exp)

            # Sample 0 stage A: dense g/phi chain first (phi gates scores),
            # then transposes; theta chunks trickle just-in-time inside the
            # stage C loop.  Later samples' stage A rides in stage C(b-1).
            start_sample(0)
            for j in range(NJ):
                proj_g_chunk(0, j)
            finish_sample_ab(0)
            for k in range(NJ):
                transpose_chunk(0, k)
            proj_t_chunk(0, 0)
            proj_t_chunk(0, 1)

            # Stage C per chunk j (PE program order): score pack t0, first
            # half of the o matmuls of chunk j-1, score pack t1 (ready right
            # as exp(t0) frees the banks), rest of o, next-sample
            # projections, paired w_o two chunks back.
            for b in range(ns):
                if b + 1 < ns:
                    start_sample(b + 1)
                state[b]["e"] = {}
                for j in range(NJ):
                    state[b]["e"][j] = ep.tile(
                        [128, KM, NC], F16, tag="e_sb", name="e_sb"
                    )
                    scores_pack(b, j, 0)
                    if j >= 1:
                        consume_o_half(b, j - 1, 0)
                    scores_pack(b, j, 1)
                    if j >= 1:
                        consume_o_half(b, j - 1, 1)
                    if j == NJ - 1:
                        consume_o_half(b, j, 0)
                    if b == 0 and j + 2 < NJ:
                        proj_t_chunk(0, j + 2)
                    if b + 1 < ns:
                        proj_g_chunk(b + 1, j)
                        transpose_chunk(b + 1, j)
                        proj_t_chunk(b + 1, j)
                    if j >= 2 and j % 2 == 0:
                        emit_wo_pair(b, j - 2)
                if b + 1 < ns:
                    finish_sample_ab(b + 1)
                consume_o_half(b, NJ - 1, 1)
                emit_wo_pair(b, NJ - 2)
                del state[b]
    nc.finalize()
    return nc


def _prep_inputs(x, w_theta, w_phi, w_g, w_o, gamma):
    xr = np.ascontiguousarray(np.asarray(x, np.float32).reshape(B, C, N))
    # projection weight: theta replicated in rows 32i:32i+16 (feeds the 4
    # PE row groups of the packed score matmuls).
    wt_full = np.zeros((C, C), np.float32)
    for i in range(4):
        wt_full[32 * i:32 * i + CT] = np.asarray(w_theta, np.float32)
    wt16 = np.ascontiguousarray(wt_full.T.astype(np.float16))  # [128, 128]
    # g rows 0:64, phi preimage rows 64:80 (32-aligned PSUM slice)
    wg_full = np.zeros((C, C), np.float32)
    wg_full[0:CG] = np.asarray(w_g, np.float32)
    wg_full[CG:CG + CT] = np.asarray(w_phi, np.float32)
    wg16 = np.ascontiguousarray(wg_full.T.astype(np.float16))  # [128, 128]
    woT = (np.float32(np.asarray(gamma).reshape(-1)[0])
           * np.asarray(w_o, np.float32)).T.astype(np.float16)  # [64, 128]
    # duplicated across partition halves for the 2-chunk row-paired matmul
    wo16 = np.ascontiguousarray(np.vstack([woT, woT]))  # [128, 128]
    return xr, wt16, wg16, wo16


def _run(x, w_theta, w_phi, w_g, w_o, gamma, trace=False):
    from concourse.bass_utils import run_bass_kernel_spmd

    xr, wt16, wg16, wo16 = _prep_inputs(x, w_theta, w_phi, w_g, w_o, gamma)
    nc = build_nc(NS)
    onec = np.zeros((C, KM, CG), np.float16)
    onec[:, :, 0] = 1.0
    ident = np.eye(CG, dtype=np.float16)
    x16 = xr.astype(np.float16)
    in_maps = [
        {"x16": np.ascontiguousarray(x16[i * NS:(i + 1) * NS]),
         "wt16": wt16, "wg16": wg16, "wo16": wo16, "onec": onec,
         "ident": ident}
        for i in range(NCORES)
    ]
    res = run_bass_kernel_spmd(nc, in_maps, list(range(NCORES)), trace=trace)
    out = np.concatenate([res.results[i]["out"] for i in range(NCORES)], axis=0)
    return out.reshape(B, C, H, W), res


def kernel(x, w_theta, w_phi, w_g, w_o, gamma):
    out, _ = _run(x, w_theta, w_phi, w_g, w_o, gamma, trace=False)
    return out


# revision 23
# speedup vs baseline: 1.1959x; 1.1959x over previous
"""Trainium2 Bass kernel for nn_Attention_51823075393746.

Self-attention block (SAGAN-style) over x:[16,128,64,64]:
  theta = w_theta @ x            [B, 16, 4096]
  phi   = pool2x2(w_phi @ x)     [B, 16, 1024]
  g     = pool2x2(w_g @ x)       [B, 64, 1024]
  beta  = softmax(theta^T phi)   [B, 4096, 1024]
  out   = gamma * (w_o @ (g @ beta^T)) + x

Sharding: data-parallel over batch, 2 samples per core on 8 cores.

Key structure (v2):
  - The K=16 score matmuls are packed 4-wide into the PE's 32-row groups
    via tile_position=(32i, 0): theta is materialized 4x (replicated
    through a padded projection weight, free), phi is pooled once and
    partition-replicated by 3 SBUF->SBUF DMAs.  The 4 packed matmuls run
    concurrently in the array (~3x measured on HW for K=32 row tiling).
  - exp() is the hard floor: 4.2M elements/sample through the one ACT
    engine at 1 elem/cycle/lane.  Each ACTIVATE spans a full 4-bank
    score pack ([128, 2048]) to amortize the (N+352)-cycle overhead, and
    nothing else is scheduled on ACT.
  - softmax uses a compile-time constant shift (exp(score - K)) and the
    denominator comes from a ones-column inside the accumulating
    g^T @ E matmul; normalization (1/s) is applied to the [64, NC]
    pooled-value tensor o before w_o (it commutes), via a partition-
    broadcast DMA of the reciprocal row.
  - residual is added on-chip from the fp16 copy of x (DVE), so x is
    loaded once (fp16) and out written once: ~6 MB DMA/core vs 18.
  - maxpools split across DVE (phi) and GpSimd (g); PSUM->SBUF copies on
    DVE; w_o matmuls deferred 2 chunks so the PE never waits on the
    reciprocal-broadcast chain.
"""

import sys

for _p in ("/opt/trn_rl_repo",):
    if _p not in sys.path:
        sys.path.insert(0, _p)

import numpy as np

import concourse.bass as bass
import concourse.bacc as bacc
import concourse.mybir as mybir
import concourse.tile as tile

F32 = mybir.dt.float32
F16 = mybir.dt.float16
AF = mybir.ActivationFunctionType
ALU = mybir.AluOpType

B, C, H, W = 16, 128, 64, 64
N = H * W          # 4096 spatial positions
M = N // 4         # 1024 pooled positions
CT = 16            # theta/phi channels (C//8)
CG = 64            # g channels (C//2)
NCORES = 8
NS = B // NCORES   # samples per core
NC = 512           # spatial chunk (free dim of matmuls)
NJ = N // NC       # 8 chunks
KM = M // 128      # 8 m-tiles of pooled positions
PR = NC // 4       # pooled positions produced per chunk (128)
K_SHIFT = 5.0      # constant softmax shift: exp(score - K) keeps fp16 happy
PACKED = True      # 4-way row-group packing of the score matmuls


def build_nc(ns: int = NS) -> bass.Bass:
    nc = bacc.Bacc()
    x16d = nc.dram_tensor("x16", [ns, C, N], F16, kind="ExternalInput")
    wt = nc.dram_tensor("wt16", [C, C], F16, kind="ExternalInput")
    wg = nc.dram_tensor("wg16", [C, C], F16, kind="ExternalInput")
    wo = nc.dram_tensor("wo16", [C, C], F16, kind="ExternalInput")
    onec = nc.dram_tensor("onec", [C, KM, CG], F16, kind="ExternalInput")
    identd = nc.dram_tensor("ident", [CG, CG], F16, kind="ExternalInput")
    out = nc.dram_tensor("out", [ns, C, N], F32, kind="ExternalOutput")

    with tile.TileContext(nc) as tc:
        with (
            tc.tile_pool(name="const", bufs=1) as const,
            tc.tile_pool(name="xp", bufs=2) as xp,
            tc.tile_pool(name="tpg", bufs=2) as tpg,
            tc.tile_pool(name="ep", bufs=3) as ep,
            tc.tile_pool(name="osb", bufs=6) as osb,
            tc.tile_pool(name="rbp", bufs=4) as rbp,
            tc.tile_pool(name="outp", bufs=4) as outp,
            tc.tile_pool(name="ps_att", bufs=1, space="PSUM") as ps_att,
            tc.tile_pool(name="ps_o", bufs=2, space="PSUM") as ps_o,
            tc.tile_pool(name="ps_x", bufs=2, space="PSUM") as ps_x,
        ):
            wg_sb = const.tile([C, C], F16)
            nc.sync.dma_start(wg_sb[:], wg[:])
            wt_sb = const.tile([C, C], F16)
            nc.sync.dma_start(wt_sb[:], wt[:])
            wo_sb = const.tile([C, C], F16)
            nc.sync.dma_start(wo_sb[:], wo[:])
            ident = const.tile([CG, CG], F16)
            nc.sync.dma_start(ident[:], identd[:])
            kbias = const.tile([C, 1], F32)
            nc.vector.memset(kbias[:], -K_SHIFT)

            # per-sample state, built by stage A/B, consumed by stage C
            state = {}

            def load_x16(b):
                x16 = xp.tile([C, N], F16, tag="x16", name="x16sb")
                for j in range(NJ):
                    nc.gpsimd.dma_start(
                        x16[:, j * NC:(j + 1) * NC],
                        x16d[b][:, j * NC:(j + 1) * NC],
                    )
                return x16

            def proj_g_chunk(b, j):
                """g+phi projection, merged maxpool, and the g^T transpose
                for m-tile j (g cols j*128 are final once chunk j pools)."""
                st = state[b]
                x16 = st["x16"]
                gphi = st["gphi"]
                sl = slice(j * NC, (j + 1) * NC)

                # g (rows 0:64) + phi preimage (rows 64:80; PSUM partition
                # slices must start 32-aligned, so phi rides here, not in
                # the theta matmul).  Rows 80:96 are zero weight filler so
                # the single merged pool reads a 96-partition slice.
                pp2 = ps_x.tile([C, NC], F32, tag="pp", name="pp2")
                nc.tensor.matmul(
                    pp2[:], lhsT=wg_sb[:], rhs=x16[:, sl], start=True, stop=True
                )
                # 2x2 maxpool: chunk j covers h rows 8j..8j+7; pooled cols
                # [PR*j, PR*(j+1)) laid out (r w): r = pooled h row (4),
                # w = pooled w col (32).
                v = pp2[0:96].rearrange(
                    "p (r a w b) -> p r w a b", r=4, a=2, b=2
                )
                po = gphi[0:96, j * PR:(j + 1) * PR].rearrange(
                    "p (r w) -> p r w", r=4
                )
                nc.vector.tensor_reduce(po, v, axis=mybir.AxisListType.XY, op=ALU.max)

            def transpose_chunk(b, k):
                """g^T for m-tile k -> ga[:, k, 0:CG]."""
                st = state[b]
                pt = ps_x.tile([C, NC], F16, tag="pp", name="pt")
                nc.tensor.transpose(
                    pt[:, 0:CG], st["gphi"][0:CG, k * 128:(k + 1) * 128],
                    ident[:],
                )
                nc.vector.tensor_copy(st["ga"][:, k, 0:CG], pt[:, 0:CG])

            def proj_t_chunk(b, j):
                """theta x4 projection (rows 32i:32i+16; other rows 0)."""
                st = state[b]
                sl = slice(j * NC, (j + 1) * NC)
                pp = ps_x.tile([C, NC], F32, tag="pp", name="pp")
                nc.tensor.matmul(
                    pp[:], lhsT=wt_sb[:], rhs=st["x16"][:, sl],
                    start=True, stop=True,
                )
                nc.vector.tensor_copy(st["thph"][:, sl], pp[:])

            def start_sample(b):
                st = state[b] = {}
                st["x16"] = load_x16(b)
                st["thph"] = tpg.tile([C, N], F16, tag="thph", name="thph")
                # rows 0:64 = g, rows 64:80 = pooled phi (row group 2 reads
                # phi from here directly), rows 80:96 zero filler
                st["gphi"] = tpg.tile([96, M], F16, tag="gphi", name="gphi")
                # phi replicas for row groups 0, 1, 3
                st["phi_rep"] = tpg.tile([128, M], F16, tag="phi_rep",
                                         name="phi_rep")
                ga = st["ga"] = tpg.tile([128, KM, 2 * CG], F16, tag="ga", name="ga")
                # cols CG:2CG: ones at col CG (softmax denominator), 0 after
                nc.gpsimd.dma_start(ga[:, :, CG:], onec[:])

            def finish_sample_ab(b):
                """phi partition-replication (after all chunks pooled)."""
                st = state[b]
                for r in (0, 1, 3):
                    nc.sync.dma_start(
                        st["phi_rep"][32 * r:32 * r + CT, :],
                        st["gphi"][CG:CG + CT, :],
                    )

            def scores_pack(b, j, t):
                """One 4-way row-group-packed score pack + its exp."""
                st = state[b]
                thph, gphi, phi_rep = st["thph"], st["gphi"], st["phi_rep"]
                sl = slice(j * NC, (j + 1) * NC)
                e_sb = st["e"][j]
                ps4 = ps_att.tile([128, 4, NC], F32, tag="s4", name="ps4")
                for i in range(4):
                    k = 4 * t + i
                    r = 32 * i if PACKED else 0
                    if PACKED and i == 2:
                        lhsT = gphi[CG:CG + CT, k * 128:(k + 1) * 128]
                    else:
                        lhsT = phi_rep[r:r + CT, k * 128:(k + 1) * 128]
                    nc.tensor.matmul(
                        ps4[:, i, :],
                        lhsT=lhsT,
                        rhs=thph[r:r + CT, sl],
                        start=True,
                        stop=True,
                        tile_position=(r, 0),
                    )
                nc.scalar.activation(
                    e_sb[:, 4 * t:4 * t + 4, :], ps4[:], AF.Exp, bias=kbias[:]
                )

            def consume_o_half(b, j, half):
                """Accumulated g^T @ E matmul for chunk j (split in two
                emission halves so score pack t1 sits between them in the
                PE queue); normalization rides with the second half."""
                st = state[b]
                ga = st["ga"]
                if half == 0:
                    st["po"] = ps_o.tile([C, NC], F32, tag="po", name="po")
                po = st["po"]
                e_sb = st["e"][j]
                ks = range(KM // 2) if half == 0 else range(KM // 2, KM)
                for k in ks:
                    nc.tensor.matmul(
                        po[:],
                        lhsT=ga[:, k, :],
                        rhs=e_sb[:, k, :],
                        start=(k == 0),
                        stop=(k == KM - 1),
                    )
                if half == 0:
                    return
                st["e"].pop(j)
                # rows 0:CG = g @ E, row CG = sum_m E = softmax denominator.
                # The bitwise-NOT seed of the fast reciprocal misreads PSUM
                # on HW -- stage the row through SBUF first.
                srow = rbp.tile([1, NC], F32, tag="srow", name="srow")
                nc.vector.tensor_copy(srow[:], po[CG:CG + 1, :])
                rrow = rbp.tile([1, NC], F32, tag="rrow", name="rrow")
                nc.vector.reciprocal_approx_fast(rrow[:], srow[:])
                rb = rbp.tile([CG, NC], F32, tag="rb", name="rb")
                nc.gpsimd.partition_broadcast(rb[:], rrow[0:1, :])
                # even chunks land at partitions 0:64, odd at 64:128 so a
                # pair of w_o matmuls can run packed in row groups {0,1} and
                # {2,3} concurrently
                r0 = (j % 2) * CG
                o_sb = osb.tile([128, NC], F16, tag="o_sb", name="o_sb")
                nc.vector.tensor_tensor(
                    o_sb[r0:r0 + CG, :], po[0:CG, :], rb[:], ALU.mult
                )
                st.setdefault("o", {})[j] = o_sb

            def emit_wo_pair(b, je):
                """w_o for chunks (je, je+1), packed into PE row groups
                {0,1} / {2,3}; then the residual add and the output DMA."""
                st = state[b]
                pfs = []
                for j in (je, je + 1):
                    r0 = (j % 2) * CG
                    pf = ps_x.tile([C, NC], F32, tag="pp", name="pf")
                    nc.tensor.matmul(
                        pf[:],
                        lhsT=wo_sb[r0:r0 + CG, :],
                        rhs=st["o"].pop(j)[r0:r0 + CG, :],
                        start=True, stop=True,
                        tile_position=(r0, 0),
                    )
                    pfs.append(pf)
                for j, pf in zip((je, je + 1), pfs):
                    sl = slice(j * NC, (j + 1) * NC)
                    o3 = outp.tile([C, NC], F32, tag="o3", name="o3")
                    nc.vector.tensor_tensor(
                        o3[:], pf[:], st["x16"][:, sl], ALU.add
                    )
                    nc.gpsimd.dma_start(out[b][:, sl], o3[:])

            # ---- schedule ----
            # Prepay the exp ACT table load (~2.7us) during startup.
            warm = rbp.tile([C, 1], F32, tag="warm", name="warm")
            nc.scalar.activation(warm[:], kbias[:], AF.Exp)

            # Sample 0 stage A: dense g/phi chain first (phi gates scores),
            # then transposes; theta chunks trickle just-in-time inside the
            # stage C loop.  Later samples' stage A rides in stage C(b-1).
            start_sample(0)
            for j in range(NJ):
                proj_g_chunk(0, j)
            finish_sample_ab(0)
            for k in range(NJ):
                transpose_chunk(0, k)
            proj_t_chunk(0, 0)
            proj_t_chunk(0, 1)

            # Stage C per chunk j (PE program order): score pack t0, first
            # half of the o matmuls of chunk j-1, score pack t1 (ready right
            # as exp(t0) frees the banks), rest of o, next-sample
            # projections, paired w_o two chunks back.
            for b in range(ns):
                if b + 1 < ns:
                    start_sample(b + 1)
                state[b]["e"] = {}
                for j in range(NJ):
                    state[b]["e"][j] = ep.tile(
                        [128, KM, NC], F16, tag="e_sb", name="e_sb"
                    )
                    scores_pack(b, j, 0)
                    if j >= 1:
                        consume_o_half(b, j - 1, 0)
                    scores_pack(b, j, 1)
                    if j >= 1:
                        consume_o_half(b, j - 1, 1)
                    if j == NJ - 1:
                        consume_o_half(b, j, 0)
                    if b == 0 and j + 2 < NJ:
                        proj_t_chunk(0, j + 2)
                    if b + 1 < ns:
                        proj_g_chunk(b + 1, j)
                        transpose_chunk(b + 1, j)
                        proj_t_chunk(b + 1, j)
                        if j == NJ - 1:
                            finish_sample_ab(b + 1)
                    if j >= 2 and j % 2 == 0:
                        emit_wo_pair(b, j - 2)
                consume_o_half(b, NJ - 1, 1)
                emit_wo_pair(b, NJ - 2)
                del state[b]
    nc.finalize()
    return nc


def _prep_inputs(x, w_theta, w_phi, w_g, w_o, gamma):
    xr = np.ascontiguousarray(np.asarray(x, np.float32).reshape(B, C, N))
    # projection weight: theta replicated in rows 32i:32i+16 (feeds the 4
    # PE row groups of the packed score matmuls).
    wt_full = np.zeros((C, C), np.float32)
    for i in range(4):
        wt_full[32 * i:32 * i + CT] = np.asarray(w_theta, np.float32)
    wt16 = np.ascontiguousarray(wt_full.T.astype(np.float16))  # [128, 128]
    # g rows 0:64, phi preimage rows 64:80 (32-aligned PSUM slice)
    wg_full = np.zeros((C, C), np.float32)
    wg_full[0:CG] = np.asarray(w_g, np.float32)
    wg_full[CG:CG + CT] = np.asarray(w_phi, np.float32)
    wg16 = np.ascontiguousarray(wg_full.T.astype(np.float16))  # [128, 128]
    woT = (np.float32(np.asarray(gamma).reshape(-1)[0])
           * np.asarray(w_o, np.float32)).T.astype(np.float16)  # [64, 128]
    # duplicated across partition halves for the 2-chunk row-paired matmul
    wo16 = np.ascontiguousarray(np.vstack([woT, woT]))  # [128, 128]
    return xr, wt16, wg16, wo16


def _run(x, w_theta, w_phi, w_g, w_o, gamma, trace=False):
    from concourse.bass_utils import run_bass_kernel_spmd

    xr, wt16, wg16, wo16 = _prep_inputs(x, w_theta, w_phi, w_g, w_o, gamma)
    nc = build_nc(NS)
    onec = np.zeros((C, KM, CG), np.float16)
    onec[:, :, 0] = 1.0
    ident = np.eye(CG, dtype=np.float16)
    x16 = xr.astype(np.float16)
    in_maps = [
        {"x16": np.ascontiguousarray(x16[i * NS:(i + 1) * NS]),
         "wt16": wt16, "wg16": wg16, "wo16": wo16, "onec": onec,
         "ident": ident}
        for i in range(NCORES)
    ]
    res = run_bass_kernel_spmd(nc, in_maps, list(range(NCORES)), trace=trace)
    out = np.concatenate([res.results[i]["out"] for i in range(NCORES)], axis=0)
    return out.reshape(B, C, H, W), res


def kernel(x, w_theta, w_phi, w_g, w_o, gamma):
    out, _ = _run(x, w_theta, w_phi, w_g, w_o, gamma, trace=False)
    return out
